# revision 38
# baseline (speedup 1.0000x reference)
"""Gaussian-splat blend kernel for 8 TRN2 NeuronCores.

Math (per pixel p, gaussians sorted nearest-first):
  q_g(p)   = (x_p - mu2d_g)^T inv_g (x_p - mu2d_g)      quadratic in x
  a_g(p)   = w_g * exp(-q/2),  w_g = sp/(1+sp), sp = softplus(alpha)
  out_c(p) = sum_g a_g * prod_{j>g}(1-a_j) * color_gc + prod_all(1-a_j)*bg_c

Device mapping (G=128 on partitions, pixels on free dim; 8-way pixel shard).
Supersteps are 1024-pixel PSUM tiles (zs, 2 banks, 4-deep pipeline),
processed in pairs sharing one feature DMA and one 2048-wide ln:
  mm1 x2/superstep (bf16, C=18): zs[:, t] = C18^T @ F18[:, t]  z=-q/2+ln w
     C18 = [ch; ch; cl], F18 = [fh; fl; fh] is the error-compensated
     bf16 split of the fp32 quadratic-coefficient matmul (plain bf16
     fails: coefficients reach ~8e3 and cancellation amplifies rounding);
     the two mm1s sit in different PE row-strips and overlap.
  ACT: a = exp(zs)            [128, 1024] per superstep (PSUM-limited)
  ACT: l = ln(1 - a) -> bf16  [128, 2048] per pair
  mm2 x2 (bf16, C=128): zs[:, t] += tri^T @ l[:, t]   (strict lower-tri)
  ACT: w = exp(zs) -> bf16    [128, 1024]             w = a * t_excl
  mm3 x2 (bf16): zs[0:3, t] = colmb^T @ w[:, t]   (into freed zs rows)
  DVE copy [3, 1024] -> SBUF, DMA out.
Host adds bg_c and reassembles [B,N,3]. ScalarE is the bottleneck
(~98.7us busy at 97% occupancy; 3 transcendental passes x 32768
cols/core @1.2GHz = 82us floor).
"""

import numpy as np
import ml_dtypes

import concourse.bass as bass
import concourse.bacc as bacc
import concourse.mybir as mybir
import concourse.tile as tile
from concourse.bass_utils import run_bass_kernel_spmd

G = 128
B = 4
N = 65536
BN = B * N
NCORES = 8
PPC = BN // NCORES          # pixels per core = 32768
SUP = 1024                  # feature DMA block (covers 2048 px, packed x2)
SUP2 = 2048                 # superstep (one 4-bank PSUM tile)
TILE = 512                  # matmul free-dim tile (one PSUM bank)

F32 = mybir.dt.float32
BF16 = mybir.dt.bfloat16
AFT = mybir.ActivationFunctionType
BF = ml_dtypes.bfloat16

PROFILE = False
LAST_EXEC_NS = None
LAST_RESULTS = None

_cached = None


def _patch_act_tables():
    """Force every activation onto the one table set that has BOTH Exp and
    Ln ("natural_log_exp_and_others") — otherwise the table-load pass
    alternates sets and burns ~1.3us per ACT_TABLE_LOAD, once per tile."""
    if getattr(bacc, "_act_tables_patched", False):
        return
    orig = bacc.get_activation_tables

    def only_nle(arch):
        tabs = orig(arch)
        return {
            name: (fns if name == "natural_log_exp_and_others" else set())
            for name, fns in tabs.items()
        }

    bacc.get_activation_tables = only_nle
    bacc._act_tables_patched = True


def _build():
    _patch_act_tables()
    nc = bacc.Bacc("TRN2", target_bir_lowering=False, debug=False,
                   num_devices=NCORES)
    # f18p: packed features — rows 0:18 = even 512-tiles, rows 32:50 = odd
    # 512-tiles, so two mm1s land in different PE row-strips and overlap.
    f18p = nc.dram_tensor("f18p", [64, PPC // 2], BF16, kind="ExternalInput")
    c18 = nc.dram_tensor("c18", [64, G], BF16, kind="ExternalInput")
    trit = nc.dram_tensor("trit", [G, G], BF16, kind="ExternalInput")
    colmb = nc.dram_tensor("colmb", [G, 3], BF16, kind="ExternalInput")
    out = nc.dram_tensor("out", [3, PPC], F32, kind="ExternalOutput")

    with tile.TileContext(nc) as tc:
        with (
            tc.tile_pool(name="const", bufs=1) as constp,
            tc.tile_pool(name="featp", bufs=4) as featp,
            tc.tile_pool(name="zs", bufs=2, space="PSUM") as zp,
            tc.tile_pool(name="ap", bufs=4) as ap_,
            tc.tile_pool(name="lp", bufs=4) as lp,
            tc.tile_pool(name="wp", bufs=4) as wp,
            tc.tile_pool(name="obuf", bufs=4) as obufp,
        ):
            c18_t = constp.tile([64, G], BF16)
            nc.sync.dma_start(c18_t[:], c18[:])
            fbufs = [featp.tile([64, SUP], BF16, tag="fbuf", name=f"fbuf{i}")
                     for i in range(PPC // SUP2)]
            # first feature block right after the mm1 constants, so mm1 can
            # start while the remaining constants stream in on another queue
            nc.sync.dma_start(fbufs[0][:], f18p[:, bass.ts(0, SUP)])
            tri_t = constp.tile([G, G], BF16)
            nc.gpsimd.dma_start(tri_t[:], trit[:])
            colmb_t = constp.tile([G, 3], BF16)
            nc.gpsimd.dma_start(colmb_t[:], colmb[:])



            # superstep = 2048 px = one 4-bank PSUM tile, all ACT passes
            # 2048-wide, deep SBUF buffering to keep the 2-deep PSUM
            # pipeline fed
            for p in range(PPC // SUP2):
                fbuf = fbufs[p]
                if p > 0:
                    nc.sync.dma_start(fbuf[:], f18p[:, bass.ts(p, SUP)])
                zs = zp.tile([G, SUP2], F32)
                for s in range(2):
                    nc.tensor.matmul(
                        zs[:, bass.ds(2 * s * TILE, TILE)], c18_t[0:18, :],
                        fbuf[0:18, bass.ts(s, TILE)], start=True, stop=False)
                    nc.tensor.matmul(
                        zs[:, bass.ds((2 * s + 1) * TILE, TILE)],
                        c18_t[32:50, :],
                        fbuf[32:50, bass.ts(s, TILE)], start=True, stop=False)
                a2 = ap_.tile([G, SUP2], F32)
                nc.scalar.activation(a2[:], zs[:], AFT.Exp)
                l2 = lp.tile([G, SUP2], BF16)
                nc.scalar.activation(l2[:], a2[:], AFT.Ln,
                                     bias=1.0, scale=-1.0)
                for i in range(4):
                    nc.tensor.matmul(
                        zs[:, bass.ts(i, TILE)], tri_t[:],
                        l2[:, bass.ts(i, TILE)], start=False, stop=True)
                w = wp.tile([G, SUP2], BF16)
                nc.scalar.activation(w[:], zs[:], AFT.Exp)
                for i in range(4):
                    nc.tensor.matmul(
                        zs[0:3, bass.ts(i, TILE)], colmb_t[:],
                        w[:, bass.ts(i, TILE)], start=True, stop=True)
                ob = obufp.tile([3, SUP2], F32)
                nc.vector.tensor_copy(ob[:], zs[0:3, :])
                nc.sync.dma_start(out[:, bass.ts(p, SUP2)], ob[:])

    nc.compile()
    return nc


def _host_prep(mu, chol, alpha, rgb, rotation, translation, projection, bg):
    # sort by camera distance in fp32 (matches reference argsort exactly)
    d32 = (mu.astype(np.float32) - translation.astype(np.float32)[None, :])
    dist = np.sqrt(np.sum(d32 * d32, axis=-1, dtype=np.float32))
    order = np.argsort(dist, kind="stable")
    mu = mu.astype(np.float64)[order]
    chol = chol.astype(np.float64)[order]
    alpha = alpha.astype(np.float64)[order]
    rgb = rgb.astype(np.float64)[order]
    rotation = rotation.astype(np.float64)
    translation = translation.astype(np.float64)
    projection = projection.astype(np.float64)
    bg = bg.astype(np.float64)

    inv_rot = rotation.T
    inv_trans = -inv_rot @ translation
    Lg = np.tril(chol) + 0.3 * np.eye(3)
    Sigma = np.einsum("gij,gkj->gik", Lg, Lg)
    mu_cam = np.einsum("ij,gj->gi", inv_rot, mu) + inv_trans
    mu2d = np.einsum("ij,gj->gi", projection, mu_cam)
    P_cam = projection @ inv_rot
    S2 = np.einsum("ij,gjk,lk->gil", P_cam, Sigma, P_cam) + 1e-4 * np.eye(2)
    det = S2[:, 0, 0] * S2[:, 1, 1] - S2[:, 0, 1] * S2[:, 1, 0]
    inv = np.empty((G, 2, 2))
    inv[:, 0, 0] = S2[:, 1, 1]
    inv[:, 0, 1] = -S2[:, 0, 1]
    inv[:, 1, 0] = -S2[:, 1, 0]
    inv[:, 1, 1] = S2[:, 0, 0]
    inv /= det[:, None, None]

    sp_ = np.logaddexp(0.0, alpha)
    wg = sp_ / (1.0 + sp_)
    color = rgb / (1.0 + np.abs(rgb))

    A = inv[:, 0, 0]
    Bc = inv[:, 0, 1] + inv[:, 1, 0]
    C = inv[:, 1, 1]
    m0, m1 = mu2d[:, 0], mu2d[:, 1]
    D = -2 * A * m0 - Bc * m1
    E = -Bc * m0 - 2 * C * m1
    F = A * m0 ** 2 + Bc * m0 * m1 + C * m1 ** 2
    coeffs = -0.5 * np.stack([A, Bc, C, D, E, F], axis=1)  # [G, 6]
    coeffs[:, 5] += np.log(wg)

    coefT = np.ascontiguousarray(coeffs.T).astype(np.float32)        # [6, G]
    ch = coefT.astype(BF)
    cl = (coefT - ch.astype(np.float32)).astype(BF)
    c18 = np.concatenate([ch, ch, cl], axis=0)                       # [18, G]
    c18p = np.zeros((64, G), BF)
    c18p[0:18] = c18
    c18p[32:50] = c18

    tri = np.tril(np.ones((G, G), np.float32), -1).astype(BF)
    colmb = (color - bg[None, :]).astype(BF)                          # [G, 3]
    return c18p, tri, colmb, bg.astype(np.float32)


def kernel(x, mu, chol, alpha, rgb, rotation, translation, projection,
           background_color):
    global _cached, LAST_EXEC_NS, LAST_RESULTS
    x = np.asarray(x, np.float32)
    c18p, tri, colmb, bg = _host_prep(
        np.asarray(mu), np.asarray(chol), np.asarray(alpha), np.asarray(rgb),
        np.asarray(rotation), np.asarray(translation), np.asarray(projection),
        np.asarray(background_color))

    xf = x.reshape(BN, 2).astype(np.float64)
    feat = np.empty((6, BN), np.float32)
    feat[0] = xf[:, 0] ** 2
    feat[1] = xf[:, 0] * xf[:, 1]
    feat[2] = xf[:, 1] ** 2
    feat[3] = xf[:, 0]
    feat[4] = xf[:, 1]
    feat[5] = 1.0
    fh = feat.astype(BF)
    fl = (feat - fh.astype(np.float32)).astype(BF)
    f18 = np.concatenate([fh, fl, fh], axis=0)                       # [18, BN]

    if _cached is None:
        _cached = _build()
    nc = _cached

    in_maps = []
    for k in range(NCORES):
        fc = f18[:, k * PPC:(k + 1) * PPC].reshape(18, PPC // TILE, TILE)
        f18p = np.zeros((64, PPC // 2), BF)
        f18p[0:18] = fc[:, 0::2].reshape(18, PPC // 2)
        f18p[32:50] = fc[:, 1::2].reshape(18, PPC // 2)
        in_maps.append({
            "f18p": f18p,
            "c18": c18p,
            "trit": tri,
            "colmb": colmb,
        })

    kwargs = {}
    if PROFILE:
        kwargs = dict(trace=True)
    res = run_bass_kernel_spmd(nc, in_maps, core_ids=list(range(NCORES)),
                               **kwargs)
    LAST_EXEC_NS = res.exec_time_ns
    LAST_RESULTS = res
    outp = np.concatenate([res.results[k]["out"] for k in range(NCORES)],
                          axis=1)                                    # [3, BN]
    return (outp.T.reshape(B, N, 3) + bg[None, None, :]).astype(np.float32)


# revision 39
# speedup vs baseline: 1.2919x; 1.2919x over previous
"""Gaussian-splat blend kernel for 8 TRN2 NeuronCores.

Math (per pixel p, gaussians sorted nearest-first):
  q_g(p)   = (x_p - mu2d_g)^T inv_g (x_p - mu2d_g)      quadratic in x
  a_g(p)   = w_g * exp(-q/2),  w_g = sp/(1+sp), sp = softplus(alpha)
  out_c(p) = sum_g a_g * prod_{j>g}(1-a_j) * color_gc + prod_all(1-a_j)*bg_c

Device mapping (G=128 on partitions, pixels on free dim; 8-way pixel shard).
Supersteps are 1024-pixel PSUM tiles (zs, 2 banks, 4-deep pipeline),
processed in pairs sharing one feature DMA and one 2048-wide ln:
  mm1 x2/superstep (bf16, C=18): zs[:, t] = C18^T @ F18[:, t]  z=-q/2+ln w
     C18 = [ch; ch; cl], F18 = [fh; fl; fh] is the error-compensated
     bf16 split of the fp32 quadratic-coefficient matmul (plain bf16
     fails: coefficients reach ~8e3 and cancellation amplifies rounding);
     the two mm1s sit in different PE row-strips and overlap.
  ACT: a = exp(zs)            [128, 1024] per superstep (PSUM-limited)
  ACT: l = ln(1 - a) -> bf16  [128, 2048] per pair
  mm2 x2 (bf16, C=128): zs[:, t] += tri^T @ l[:, t]   (strict lower-tri)
  ACT: w = exp(zs) -> bf16    [128, 1024]             w = a * t_excl
  mm3 x2 (bf16): zs[0:3, t] = colmb^T @ w[:, t]   (into freed zs rows)
  DVE copy [3, 1024] -> SBUF, DMA out.
Host adds bg_c and reassembles [B,N,3]. ScalarE is the bottleneck
(~98.7us busy at 97% occupancy; 3 transcendental passes x 32768
cols/core @1.2GHz = 82us floor).
"""

import numpy as np
import ml_dtypes

import concourse.bass as bass
import concourse.bacc as bacc
import concourse.mybir as mybir
import concourse.tile as tile
from concourse.bass_utils import run_bass_kernel_spmd

G = 128
B = 4
N = 65536
BN = B * N
NCORES = 8
PPC = BN // NCORES          # pixels per core = 32768
SUP = 1024                  # feature DMA block (covers 2048 px, packed x2)
SUP2 = 2048                 # superstep (one 4-bank PSUM tile)
TILE = 512                  # matmul free-dim tile (one PSUM bank)

F32 = mybir.dt.float32
BF16 = mybir.dt.bfloat16
AFT = mybir.ActivationFunctionType
BF = ml_dtypes.bfloat16

PROFILE = False
LAST_EXEC_NS = None
LAST_RESULTS = None

_cached = None


def _patch_act_tables():
    """Force every activation onto the one table set that has BOTH Exp and
    Ln ("natural_log_exp_and_others") — otherwise the table-load pass
    alternates sets and burns ~1.3us per ACT_TABLE_LOAD, once per tile."""
    if getattr(bacc, "_act_tables_patched", False):
        return
    orig = bacc.get_activation_tables

    def only_nle(arch):
        tabs = orig(arch)
        return {
            name: (fns if name == "natural_log_exp_and_others" else set())
            for name, fns in tabs.items()
        }

    bacc.get_activation_tables = only_nle
    bacc._act_tables_patched = True


def _build():
    _patch_act_tables()
    nc = bacc.Bacc("TRN2", target_bir_lowering=False, debug=False,
                   num_devices=NCORES)
    # f18p: packed features — rows 0:18 = even 512-tiles, rows 32:50 = odd
    # 512-tiles, so two mm1s land in different PE row-strips and overlap.
    f18p = nc.dram_tensor("f18p", [64, PPC // 2], BF16, kind="ExternalInput")
    c18 = nc.dram_tensor("c18", [64, G], BF16, kind="ExternalInput")
    trit = nc.dram_tensor("trit", [G, G], BF16, kind="ExternalInput")
    colmb = nc.dram_tensor("colmb", [G, 3], BF16, kind="ExternalInput")
    out = nc.dram_tensor("out", [3, PPC], F32, kind="ExternalOutput")

    with tile.TileContext(nc) as tc:
        with (
            tc.tile_pool(name="const", bufs=1) as constp,
            tc.tile_pool(name="featp", bufs=3) as featp,
            tc.tile_pool(name="zs", bufs=4, space="PSUM") as zp,
            tc.tile_pool(name="ap", bufs=3) as ap_,
            tc.tile_pool(name="lp", bufs=3) as lp,
            tc.tile_pool(name="wp", bufs=4) as wp,
            tc.tile_pool(name="obuf", bufs=4) as obufp,
        ):
            c18_t = constp.tile([64, G], BF16)
            nc.sync.dma_start(c18_t[:], c18[:])
            fbufs = [featp.tile([64, SUP], BF16, tag="fbuf", name=f"fbuf{i}")
                     for i in range(PPC // SUP2)]
            # first feature block right after the mm1 constants, so mm1 can
            # start while the remaining constants stream in on another queue
            nc.sync.dma_start(fbufs[0][:], f18p[:, bass.ts(0, SUP)])
            tri_t = constp.tile([G, G], BF16)
            nc.gpsimd.dma_start(tri_t[:], trit[:])
            colmb_t = constp.tile([G, 3], BF16)
            nc.gpsimd.dma_start(colmb_t[:], colmb[:])



            # supersteps are paired: one [64, 1024] feature DMA and one
            # 2048-wide ln per pair; exp stays 1024-wide (PSUM-limited)
            for p in range(PPC // SUP2):
                fbuf = fbufs[p]
                if p > 0:
                    nc.sync.dma_start(fbuf[:], f18p[:, bass.ts(p, SUP)])
                a2 = ap_.tile([G, SUP2], F32)
                l2 = lp.tile([G, SUP2], BF16)
                zss = []
                for s in range(2):
                    zs = zp.tile([G, SUP], F32)
                    zss.append(zs)
                    nc.tensor.matmul(
                        zs[:, 0:TILE], c18_t[0:18, :],
                        fbuf[0:18, bass.ts(s, TILE)], start=True, stop=False)
                    nc.tensor.matmul(
                        zs[:, TILE:SUP], c18_t[32:50, :],
                        fbuf[32:50, bass.ts(s, TILE)], start=True, stop=False)
                    nc.scalar.activation(a2[:, bass.ts(s, SUP)], zs[:],
                                         AFT.Exp)
                nc.scalar.activation(l2[:], a2[:], AFT.Ln,
                                     bias=1.0, scale=-1.0)
                for s in range(2):
                    base = p * SUP2 + s * SUP
                    zs = zss[s]
                    for i in range(2):
                        nc.tensor.matmul(
                            zs[:, bass.ts(i, TILE)], tri_t[:],
                            l2[:, bass.ds(s * SUP + i * TILE, TILE)],
                            start=False, stop=True)
                    w = wp.tile([G, SUP], BF16)
                    nc.scalar.activation(w[:], zs[:], AFT.Exp)
                    for i in range(2):
                        nc.tensor.matmul(
                            zs[0:3, bass.ts(i, TILE)], colmb_t[:],
                            w[:, bass.ts(i, TILE)], start=True, stop=True)
                    ob = obufp.tile([3, SUP], F32)
                    nc.vector.tensor_copy(ob[:], zs[0:3, :])
                    nc.sync.dma_start(out[:, base:base + SUP], ob[:])

    nc.compile()
    return nc


def _host_prep(mu, chol, alpha, rgb, rotation, translation, projection, bg):
    # sort by camera distance in fp32 (matches reference argsort exactly)
    d32 = (mu.astype(np.float32) - translation.astype(np.float32)[None, :])
    dist = np.sqrt(np.sum(d32 * d32, axis=-1, dtype=np.float32))
    order = np.argsort(dist, kind="stable")
    mu = mu.astype(np.float64)[order]
    chol = chol.astype(np.float64)[order]
    alpha = alpha.astype(np.float64)[order]
    rgb = rgb.astype(np.float64)[order]
    rotation = rotation.astype(np.float64)
    translation = translation.astype(np.float64)
    projection = projection.astype(np.float64)
    bg = bg.astype(np.float64)

    inv_rot = rotation.T
    inv_trans = -inv_rot @ translation
    Lg = np.tril(chol) + 0.3 * np.eye(3)
    Sigma = np.einsum("gij,gkj->gik", Lg, Lg)
    mu_cam = np.einsum("ij,gj->gi", inv_rot, mu) + inv_trans
    mu2d = np.einsum("ij,gj->gi", projection, mu_cam)
    P_cam = projection @ inv_rot
    S2 = np.einsum("ij,gjk,lk->gil", P_cam, Sigma, P_cam) + 1e-4 * np.eye(2)
    det = S2[:, 0, 0] * S2[:, 1, 1] - S2[:, 0, 1] * S2[:, 1, 0]
    inv = np.empty((G, 2, 2))
    inv[:, 0, 0] = S2[:, 1, 1]
    inv[:, 0, 1] = -S2[:, 0, 1]
    inv[:, 1, 0] = -S2[:, 1, 0]
    inv[:, 1, 1] = S2[:, 0, 0]
    inv /= det[:, None, None]

    sp_ = np.logaddexp(0.0, alpha)
    wg = sp_ / (1.0 + sp_)
    color = rgb / (1.0 + np.abs(rgb))

    A = inv[:, 0, 0]
    Bc = inv[:, 0, 1] + inv[:, 1, 0]
    C = inv[:, 1, 1]
    m0, m1 = mu2d[:, 0], mu2d[:, 1]
    D = -2 * A * m0 - Bc * m1
    E = -Bc * m0 - 2 * C * m1
    F = A * m0 ** 2 + Bc * m0 * m1 + C * m1 ** 2
    coeffs = -0.5 * np.stack([A, Bc, C, D, E, F], axis=1)  # [G, 6]
    coeffs[:, 5] += np.log(wg)

    coefT = np.ascontiguousarray(coeffs.T).astype(np.float32)        # [6, G]
    ch = coefT.astype(BF)
    cl = (coefT - ch.astype(np.float32)).astype(BF)
    c18 = np.concatenate([ch, ch, cl], axis=0)                       # [18, G]
    c18p = np.zeros((64, G), BF)
    c18p[0:18] = c18
    c18p[32:50] = c18

    tri = np.tril(np.ones((G, G), np.float32), -1).astype(BF)
    colmb = (color - bg[None, :]).astype(BF)                          # [G, 3]
    return c18p, tri, colmb, bg.astype(np.float32)


def kernel(x, mu, chol, alpha, rgb, rotation, translation, projection,
           background_color):
    global _cached, LAST_EXEC_NS, LAST_RESULTS
    x = np.asarray(x, np.float32)
    c18p, tri, colmb, bg = _host_prep(
        np.asarray(mu), np.asarray(chol), np.asarray(alpha), np.asarray(rgb),
        np.asarray(rotation), np.asarray(translation), np.asarray(projection),
        np.asarray(background_color))

    xf = x.reshape(BN, 2).astype(np.float64)
    feat = np.empty((6, BN), np.float32)
    feat[0] = xf[:, 0] ** 2
    feat[1] = xf[:, 0] * xf[:, 1]
    feat[2] = xf[:, 1] ** 2
    feat[3] = xf[:, 0]
    feat[4] = xf[:, 1]
    feat[5] = 1.0
    fh = feat.astype(BF)
    fl = (feat - fh.astype(np.float32)).astype(BF)
    f18 = np.concatenate([fh, fl, fh], axis=0)                       # [18, BN]

    if _cached is None:
        _cached = _build()
    nc = _cached

    in_maps = []
    for k in range(NCORES):
        fc = f18[:, k * PPC:(k + 1) * PPC].reshape(18, PPC // TILE, TILE)
        f18p = np.zeros((64, PPC // 2), BF)
        f18p[0:18] = fc[:, 0::2].reshape(18, PPC // 2)
        f18p[32:50] = fc[:, 1::2].reshape(18, PPC // 2)
        in_maps.append({
            "f18p": f18p,
            "c18": c18p,
            "trit": tri,
            "colmb": colmb,
        })

    kwargs = {}
    if PROFILE:
        kwargs = dict(trace=True)
    res = run_bass_kernel_spmd(nc, in_maps, core_ids=list(range(NCORES)),
                               **kwargs)
    LAST_EXEC_NS = res.exec_time_ns
    LAST_RESULTS = res
    outp = np.concatenate([res.results[k]["out"] for k in range(NCORES)],
                          axis=1)                                    # [3, BN]
    return (outp.T.reshape(B, N, 3) + bg[None, None, :]).astype(np.float32)


# revision 40
# speedup vs baseline: 1.3118x; 1.0154x over previous
"""Gaussian-splat blend kernel for 8 TRN2 NeuronCores.

Math (per pixel p, gaussians sorted nearest-first):
  q_g(p)   = (x_p - mu2d_g)^T inv_g (x_p - mu2d_g)      quadratic in x
  a_g(p)   = w_g * exp(-q/2),  w_g = sp/(1+sp), sp = softplus(alpha)
  out_c(p) = sum_g a_g * prod_{j>g}(1-a_j) * color_gc + prod_all(1-a_j)*bg_c

Device mapping (G=128 on partitions, pixels on free dim; 8-way pixel shard).
Supersteps are 1024-pixel PSUM tiles (zs, 2 banks, 4-deep pipeline),
processed in pairs sharing one feature DMA and one 2048-wide ln:
  mm1 x2/superstep (bf16, C=18): zs[:, t] = C18^T @ F18[:, t]  z=-q/2+ln w
     C18 = [ch; ch; cl], F18 = [fh; fl; fh] is the error-compensated
     bf16 split of the fp32 quadratic-coefficient matmul (plain bf16
     fails: coefficients reach ~8e3 and cancellation amplifies rounding);
     the two mm1s sit in different PE row-strips and overlap.
  ACT: a = exp(zs)            [128, 1024] per superstep (PSUM-limited)
  ACT: l = ln(1 - a) -> bf16  [128, 2048] per pair
  mm2 x2 (bf16, C=128): zs[:, t] += tri^T @ l[:, t]   (strict lower-tri)
  ACT: w = exp(zs) -> bf16    [128, 1024]             w = a * t_excl
  mm3 x2 (bf16): zs[0:3, t] = colmb^T @ w[:, t]   (into freed zs rows)
  DVE copy [3, 1024] -> SBUF, DMA out.
Host adds bg_c and reassembles [B,N,3]. ScalarE is the bottleneck
(~98.7us busy at 97% occupancy; 3 transcendental passes x 32768
cols/core @1.2GHz = 82us floor).
"""

import numpy as np
import ml_dtypes

import concourse.bass as bass
import concourse.bacc as bacc
import concourse.mybir as mybir
import concourse.tile as tile
from concourse.bass_utils import run_bass_kernel_spmd

G = 128
B = 4
N = 65536
BN = B * N
NCORES = 8
PPC = BN // NCORES          # pixels per core = 32768
SUP = 1024                  # feature DMA block (covers 2048 px, packed x2)
SUP2 = 2048                 # superstep (one 4-bank PSUM tile)
TILE = 512                  # matmul free-dim tile (one PSUM bank)

F32 = mybir.dt.float32
BF16 = mybir.dt.bfloat16
AFT = mybir.ActivationFunctionType
BF = ml_dtypes.bfloat16

PROFILE = False
LAST_EXEC_NS = None
LAST_RESULTS = None

_cached = None


def _patch_act_tables():
    """Force every activation onto the one table set that has BOTH Exp and
    Ln ("natural_log_exp_and_others") — otherwise the table-load pass
    alternates sets and burns ~1.3us per ACT_TABLE_LOAD, once per tile."""
    if getattr(bacc, "_act_tables_patched", False):
        return
    orig = bacc.get_activation_tables

    def only_nle(arch):
        tabs = orig(arch)
        return {
            name: (fns if name == "natural_log_exp_and_others" else set())
            for name, fns in tabs.items()
        }

    bacc.get_activation_tables = only_nle
    bacc._act_tables_patched = True


def _build():
    _patch_act_tables()
    nc = bacc.Bacc("TRN2", target_bir_lowering=False, debug=False,
                   num_devices=NCORES)
    # f18p: packed features — rows 0:18 = even 512-tiles, rows 32:50 = odd
    # 512-tiles, so two mm1s land in different PE row-strips and overlap.
    f18p = nc.dram_tensor("f18p", [64, PPC // 2], BF16, kind="ExternalInput")
    c18 = nc.dram_tensor("c18", [64, G], BF16, kind="ExternalInput")
    trit = nc.dram_tensor("trit", [G, G], BF16, kind="ExternalInput")
    colmb = nc.dram_tensor("colmb", [G, 3], BF16, kind="ExternalInput")
    out = nc.dram_tensor("out", [3, PPC], F32, kind="ExternalOutput")

    with tile.TileContext(nc) as tc:
        with (
            tc.tile_pool(name="const", bufs=1) as constp,
            tc.tile_pool(name="featp", bufs=3) as featp,
            tc.tile_pool(name="zs", bufs=4, space="PSUM") as zp,
            tc.tile_pool(name="ap", bufs=3) as ap_,
            tc.tile_pool(name="lp", bufs=3) as lp,
            tc.tile_pool(name="wp", bufs=4) as wp,
            tc.tile_pool(name="obuf", bufs=4) as obufp,
        ):
            # dependency-free dummy activation: pulls the ~1.3us
            # ACT_TABLE_LOAD into the idle DMA-wait head instead of behind
            # the first exp's semaphore wait
            dummy = constp.tile([1, 8], F32)
            nc.gpsimd.memset(dummy[:], 0.0)
            nc.scalar.activation(dummy[:], dummy[:], AFT.Exp)

            c18_t = constp.tile([64, G], BF16)
            nc.sync.dma_start(c18_t[:], c18[:])
            fbufs = [featp.tile([64, SUP], BF16, tag="fbuf", name=f"fbuf{i}")
                     for i in range(PPC // SUP2)]
            # first feature block right after the mm1 constants, so mm1 can
            # start while the remaining constants stream in on another queue
            nc.sync.dma_start(fbufs[0][:], f18p[:, bass.ts(0, SUP)])
            tri_t = constp.tile([G, G], BF16)
            nc.gpsimd.dma_start(tri_t[:], trit[:])
            colmb_t = constp.tile([G, 3], BF16)
            nc.gpsimd.dma_start(colmb_t[:], colmb[:])



            # supersteps are paired: one [64, 1024] feature DMA and one
            # 2048-wide ln per pair; exp stays 1024-wide (PSUM-limited)
            for p in range(PPC // SUP2):
                fbuf = fbufs[p]
                if p > 0:
                    nc.sync.dma_start(fbuf[:], f18p[:, bass.ts(p, SUP)])
                a2 = ap_.tile([G, SUP2], F32)
                l2 = lp.tile([G, SUP2], BF16)
                zss = []
                for s in range(2):
                    zs = zp.tile([G, SUP], F32)
                    zss.append(zs)
                    nc.tensor.matmul(
                        zs[:, 0:TILE], c18_t[0:18, :],
                        fbuf[0:18, bass.ts(s, TILE)], start=True, stop=False)
                    nc.tensor.matmul(
                        zs[:, TILE:SUP], c18_t[32:50, :],
                        fbuf[32:50, bass.ts(s, TILE)], start=True, stop=False)
                    nc.scalar.activation(a2[:, bass.ts(s, SUP)], zs[:],
                                         AFT.Exp)
                nc.scalar.activation(l2[:], a2[:], AFT.Ln,
                                     bias=1.0, scale=-1.0)
                for s in range(2):
                    base = p * SUP2 + s * SUP
                    zs = zss[s]
                    for i in range(2):
                        nc.tensor.matmul(
                            zs[:, bass.ts(i, TILE)], tri_t[:],
                            l2[:, bass.ds(s * SUP + i * TILE, TILE)],
                            start=False, stop=True)
                    w = wp.tile([G, SUP], BF16)
                    nc.scalar.activation(w[:], zs[:], AFT.Exp)
                    for i in range(2):
                        nc.tensor.matmul(
                            zs[0:3, bass.ts(i, TILE)], colmb_t[:],
                            w[:, bass.ts(i, TILE)], start=True, stop=True)
                    ob = obufp.tile([3, SUP], F32)
                    nc.vector.tensor_copy(ob[:], zs[0:3, :])
                    nc.sync.dma_start(out[:, base:base + SUP], ob[:])

    nc.compile()
    return nc


def _host_prep(mu, chol, alpha, rgb, rotation, translation, projection, bg):
    # sort by camera distance in fp32 (matches reference argsort exactly)
    d32 = (mu.astype(np.float32) - translation.astype(np.float32)[None, :])
    dist = np.sqrt(np.sum(d32 * d32, axis=-1, dtype=np.float32))
    order = np.argsort(dist, kind="stable")
    mu = mu.astype(np.float64)[order]
    chol = chol.astype(np.float64)[order]
    alpha = alpha.astype(np.float64)[order]
    rgb = rgb.astype(np.float64)[order]
    rotation = rotation.astype(np.float64)
    translation = translation.astype(np.float64)
    projection = projection.astype(np.float64)
    bg = bg.astype(np.float64)

    inv_rot = rotation.T
    inv_trans = -inv_rot @ translation
    Lg = np.tril(chol) + 0.3 * np.eye(3)
    Sigma = np.einsum("gij,gkj->gik", Lg, Lg)
    mu_cam = np.einsum("ij,gj->gi", inv_rot, mu) + inv_trans
    mu2d = np.einsum("ij,gj->gi", projection, mu_cam)
    P_cam = projection @ inv_rot
    S2 = np.einsum("ij,gjk,lk->gil", P_cam, Sigma, P_cam) + 1e-4 * np.eye(2)
    det = S2[:, 0, 0] * S2[:, 1, 1] - S2[:, 0, 1] * S2[:, 1, 0]
    inv = np.empty((G, 2, 2))
    inv[:, 0, 0] = S2[:, 1, 1]
    inv[:, 0, 1] = -S2[:, 0, 1]
    inv[:, 1, 0] = -S2[:, 1, 0]
    inv[:, 1, 1] = S2[:, 0, 0]
    inv /= det[:, None, None]

    sp_ = np.logaddexp(0.0, alpha)
    wg = sp_ / (1.0 + sp_)
    color = rgb / (1.0 + np.abs(rgb))

    A = inv[:, 0, 0]
    Bc = inv[:, 0, 1] + inv[:, 1, 0]
    C = inv[:, 1, 1]
    m0, m1 = mu2d[:, 0], mu2d[:, 1]
    D = -2 * A * m0 - Bc * m1
    E = -Bc * m0 - 2 * C * m1
    F = A * m0 ** 2 + Bc * m0 * m1 + C * m1 ** 2
    coeffs = -0.5 * np.stack([A, Bc, C, D, E, F], axis=1)  # [G, 6]
    coeffs[:, 5] += np.log(wg)

    coefT = np.ascontiguousarray(coeffs.T).astype(np.float32)        # [6, G]
    ch = coefT.astype(BF)
    cl = (coefT - ch.astype(np.float32)).astype(BF)
    c18 = np.concatenate([ch, ch, cl], axis=0)                       # [18, G]
    c18p = np.zeros((64, G), BF)
    c18p[0:18] = c18
    c18p[32:50] = c18

    tri = np.tril(np.ones((G, G), np.float32), -1).astype(BF)
    colmb = (color - bg[None, :]).astype(BF)                          # [G, 3]
    return c18p, tri, colmb, bg.astype(np.float32)


def kernel(x, mu, chol, alpha, rgb, rotation, translation, projection,
           background_color):
    global _cached, LAST_EXEC_NS, LAST_RESULTS
    x = np.asarray(x, np.float32)
    c18p, tri, colmb, bg = _host_prep(
        np.asarray(mu), np.asarray(chol), np.asarray(alpha), np.asarray(rgb),
        np.asarray(rotation), np.asarray(translation), np.asarray(projection),
        np.asarray(background_color))

    xf = x.reshape(BN, 2).astype(np.float64)
    feat = np.empty((6, BN), np.float32)
    feat[0] = xf[:, 0] ** 2
    feat[1] = xf[:, 0] * xf[:, 1]
    feat[2] = xf[:, 1] ** 2
    feat[3] = xf[:, 0]
    feat[4] = xf[:, 1]
    feat[5] = 1.0
    fh = feat.astype(BF)
    fl = (feat - fh.astype(np.float32)).astype(BF)
    f18 = np.concatenate([fh, fl, fh], axis=0)                       # [18, BN]

    if _cached is None:
        _cached = _build()
    nc = _cached

    in_maps = []
    for k in range(NCORES):
        fc = f18[:, k * PPC:(k + 1) * PPC].reshape(18, PPC // TILE, TILE)
        f18p = np.zeros((64, PPC // 2), BF)
        f18p[0:18] = fc[:, 0::2].reshape(18, PPC // 2)
        f18p[32:50] = fc[:, 1::2].reshape(18, PPC // 2)
        in_maps.append({
            "f18p": f18p,
            "c18": c18p,
            "trit": tri,
            "colmb": colmb,
        })

    kwargs = {}
    if PROFILE:
        kwargs = dict(trace=True)
    res = run_bass_kernel_spmd(nc, in_maps, core_ids=list(range(NCORES)),
                               **kwargs)
    LAST_EXEC_NS = res.exec_time_ns
    LAST_RESULTS = res
    outp = np.concatenate([res.results[k]["out"] for k in range(NCORES)],
                          axis=1)                                    # [3, BN]
    return (outp.T.reshape(B, N, 3) + bg[None, None, :]).astype(np.float32)
